# revision 18
# baseline (speedup 1.0000x reference)
"""Head-sharded tensor-parallel attention (2 heads/core, 8 cores).

Each core computes QKV for its 2 heads over ALL active tokens of both
batches, full attention for those heads, and a partial output
``AO_c @ W_out[c's 128 inner dims, :]``.  The host sums the 8 partial
outputs (the unshard step of the W_out-row sharding) and scatters back
into masked positions.  Masked-out rows of the reference output are
exactly zero, so only active tokens are processed (gathered on host);
pad keys have x=0 -> k=0 -> exp(0)=1, corrected by subtracting the pad
count from the softmax denominator.
"""

import math
from contextlib import ExitStack

import numpy as np
import ml_dtypes

import concourse.bass as bass
import concourse.mybir as mybir
import concourse.tile as tile
from concourse import bacc
from concourse.bass_utils import run_bass_kernel_spmd

P = 128
D = 1024          # model dim
HEADS = 16
DH = 64
VW = DH + 1       # v columns per head + ones column (softmax denominator)
SCALE = DH ** -0.5
N_CORES = 8
BF16 = mybir.dt.bfloat16
F32 = mybir.dt.float32


def _chunks(total, step):
    out = []
    o = 0
    while o < total:
        out.append((o, min(step, total - o)))
        o += step
    return out


def _build(T: int, mq0: int, mq1: int):
    """Per-core SPMD graph; T = padded key count per batch (mult of 128),
    mq0/mq1 = per-batch query counts (real tokens padded to 16)."""
    nkt = T // P
    NT = 2 * T
    MQ = (mq0, mq1)
    KCS = _chunks(T, 512)                      # key chunks (K^T free dim)
    QCS = [_chunks(mq0, 512), _chunks(mq1, 512)]   # query chunks per batch

    nc = bacc.Bacc(None, target_bir_lowering=False, num_devices=N_CORES)

    xt_in = nc.declare_dram_parameter("xt", [D, NT], BF16, isOutput=False)
    wqkv_in = nc.declare_dram_parameter("wqkv", [D, 384], BF16, isOutput=False)
    wout_in = nc.declare_dram_parameter("wout", [P, D], BF16, isOutput=False)
    npad_in = nc.declare_dram_parameter("npad", [1, 2], F32, isOutput=False)
    out_ext = nc.declare_dram_parameter("out", [NT, D], BF16, isOutput=True)

    with tile.TileContext(nc) as tc, ExitStack() as ctx:
        sb = ctx.enter_context(tc.tile_pool(name="sb", bufs=1))
        ps = ctx.enter_context(tc.tile_pool(name="ps", bufs=1, space="PSUM"))

        npad_sb = sb.tile([1, 2], F32, tag="npad", bufs=1, name="npad_sb")
        nc.sync.dma_start(npad_sb[:], npad_in[:])

        # HAM warm-up: dependency-free matmuls so the PE clock ramps while
        # the first DMAs land.
        warm = sb.tile([P, 512], BF16, tag="warm", bufs=1, name="warm")
        nc.vector.memset(warm[:], 0.0)
        for i in range(18):
            wps = ps.tile([P, 512], F32, tag="ss", bufs=2, name=f"wps{i}")
            nc.tensor.matmul(wps[:], warm[:, 0:P], warm[:],
                             start=True, stop=True, skip_group_check=True)

        # ---- input DMAs, round-robined over sequencers.
        seqs = [nc.sync, nc.scalar, nc.gpsimd]
        _n = [0]

        def dma(dst, src, seq=None):
            (seqs[_n[0] % len(seqs)] if seq is None else seq).dma_start(dst, src)
            _n[0] += 1

        wqkv_sb = []
        for kc in range(8):
            tw = sb.tile([P, 384], BF16, tag="wqkv", bufs=8, name=f"wqkv{kc}")
            dma(tw[:], wqkv_in[kc * P:(kc + 1) * P, :])
            wqkv_sb.append(tw)
        xt = []
        for kc in range(8):
            t_ = sb.tile([P, NT], BF16, tag="xt", bufs=8, name=f"xt{kc}")
            xt.append(t_)
        # batch-0 columns land chunk-by-chunk so K/Q projections (and the
        # exp stream behind them) start before the full x^T arrives.
        for qo, qw in KCS:
            for kc in range(8):
                dma(xt[kc][:, qo:qo + qw], xt_in[kc * P:(kc + 1) * P, qo:qo + qw])
        for kc in range(8):
            dma(xt[kc][:, T:NT], xt_in[kc * P:(kc + 1) * P, T:NT])
        wout_sb = sb.tile([P, D], BF16, tag="wout", bufs=1, name="wout_sb")
        dma(wout_sb[:, 0:512], wout_in[:, 0:512])
        dma(wout_sb[:, 512:D], wout_in[:, 512:D])

        # ---- K^T and Q^T per batch: [128 dims(2 heads), T] bf16 in SBUF.
        kf = [None, None]
        qt = [None, None]

        def proj_kq(b):
            kfb = sb.tile([P, T], BF16, tag="kf", bufs=2, name=f"kf{b}")
            qtb = sb.tile([P, T], BF16, tag="qt", bufs=2, name=f"qt{b}")
            # chunk-major so S(kt<qc_end, qc0) unlocks after the first pair
            jobs = []
            for ci in range(len(KCS)):
                jobs.append((kfb, 128, KCS[ci]))
                if ci < len(QCS[b]):
                    jobs.append((qtb, 0, QCS[b][ci]))
            for dst, col0, (qo, qw) in jobs:
                pps = ps.tile([P, 512], F32, tag="ss", bufs=2,
                              name=f"pp{b}_{col0}_{qo}")
                for i in range(8):
                    kc = (i + qo // 512) % 8
                    nc.tensor.matmul(
                        pps[:, 0:qw],
                        wqkv_sb[kc][:, col0:col0 + P],
                        xt[kc][:, b * T + qo: b * T + qo + qw],
                        start=(i == 0), stop=(i == 7))
                nc.vector.tensor_copy(dst[:, qo:qo + qw], pps[:, 0:qw])
            kf[b] = kfb
            qt[b] = qtb

        # ---- V tiles per (batch, kt): [128 keys, 2*VW] bf16 with ones col.
        vt = [[None] * nkt, [None] * nkt]

        def proj_v(b):
            for kt in range(nkt):
                t_ = sb.tile([P, 2 * VW], BF16, tag="vt", bufs=2 * nkt,
                             name=f"vt{b}_{kt}")
                nc.gpsimd.memset(
                    t_[:].rearrange("p (h c) -> p h c", c=VW)[:, :, DH:DH + 1], 1.0)
                vps = ps.tile([P, P], F32, tag="ss", bufs=2, name=f"vps{b}_{kt}")
                for i in range(8):
                    kc = (i + kt) % 8
                    nc.tensor.matmul(
                        vps[:],
                        xt[kc][:, b * T + kt * P: b * T + (kt + 1) * P],
                        wqkv_sb[kc][:, 256:384],
                        start=(i == 0), stop=(i == 7))
                nc.vector.tensor_copy(
                    t_[:].rearrange("p (h c) -> p h c", c=VW)[:, :, 0:DH],
                    vps[:].rearrange("p (h c) -> p h c", c=DH))
                vt[b][kt] = t_

        # ---- S + exp stream for one batch.  S^T tiles [keys, queries] per
        # (kt, qchunk); both heads share one 2-bank psum tile; exp writes a
        # per-kt SBUF tile pt[b][kt] = [128, 2*T] bf16 (head-major halves).
        pt = [[None] * nkt, [None] * nkt]

        def s_exp(b):
            for kt in range(nkt):
                pt[b][kt] = sb.tile([P, 2 * T], BF16, tag="pt", bufs=nkt + 5,
                                    name=f"pt{b}_{kt}")
            # qc-outer so AV groups for qc unlock at 1/len(QCS) stream marks
            for qo, qw in QCS[b]:
                for kt in range(nkt):
                    sps = ps.tile([P, 1024], F32, tag="sps", bufs=2,
                                  name=f"sps{b}_{kt}_{qo}")
                    for h in range(2):
                        nc.tensor.matmul(
                            sps[:, h * 512: h * 512 + qw],
                            kf[b][h * DH:(h + 1) * DH, kt * P:(kt + 1) * P],
                            qt[b][h * DH:(h + 1) * DH, qo:qo + qw],
                            start=True, stop=True, skip_group_check=True)
                    nc.scalar.activation(
                        pt[b][kt][:].rearrange("p (u c) -> p u c", c=T)[:, 0:2, qo:qo + qw],
                        sps[:].rearrange("p (u c) -> p u c", c=512)[:, 0:2, 0:qw],
                        mybir.ActivationFunctionType.Exp, scale=SCALE)

        # ---- AV pass + normalize for one batch -> aoT[b] [128, T] bf16.
        aoT = [None, None]

        def av_norm(b):
            aob = sb.tile([P, T], BF16, tag="aoT", bufs=2, name=f"aoT{b}")
            aoT[b] = aob
            av_h = [sb.tile([VW, T], F32, tag="avs", bufs=2, name=f"avs{b}_{h}")
                    for h in range(2)]
            tmpb = sb.tile([DH, T], BF16, tag="tmpb", bufs=2, name=f"tmpb{b}")
            # qc-outer to match the exp stream's completion order
            for qo, qw in QCS[b]:
                for h in range(2):
                    av_ = av_h[h]
                    avp = ps.tile([P, 512], F32, tag="av", bufs=2,
                                  name=f"avp{b}_{h}_{qo}")
                    for kt in range(nkt):
                        nc.tensor.matmul(
                            avp[0:VW, 0:qw],
                            vt[b][kt][:, h * VW:(h + 1) * VW],
                            pt[b][kt][:, h * T + qo: h * T + qo + qw],
                            start=(kt == 0), stop=(kt == nkt - 1),
                            skip_group_check=True)
                    nc.vector.tensor_copy(av_[:, qo:qo + qw], avp[0:VW, 0:qw])
                    # chunked normalize chain so out-proj tiles unlock early
                    den = sb.tile([1, 512], F32, tag="den", bufs=4,
                                  name=f"den{b}_{h}_{qo}")
                    nc.vector.tensor_scalar(den[:, 0:qw], av_[DH:DH + 1, qo:qo + qw],
                                            npad_sb[0:1, b:b + 1], None,
                                            op0=mybir.AluOpType.subtract)
                    rec = sb.tile([1, 512], F32, tag="rec", bufs=4,
                                  name=f"rec{b}_{h}_{qo}")
                    nc.vector.reciprocal_approx_fast(rec[:, 0:qw], den[:, 0:qw])
                    fac = sb.tile([DH, 512], F32, tag="fac", bufs=2,
                                  name=f"fac{b}_{h}_{qo}")
                    nc.gpsimd.partition_broadcast(fac[:, 0:qw], rec[:, 0:qw])
                    if h == 0:
                        nc.vector.tensor_tensor(aob[0:DH, qo:qo + qw],
                                                av_[0:DH, qo:qo + qw],
                                                fac[:, 0:qw],
                                                op=mybir.AluOpType.mult)
                    else:
                        nc.vector.tensor_tensor(tmpb[:, qo:qo + qw],
                                                av_[0:DH, qo:qo + qw],
                                                fac[:, 0:qw],
                                                op=mybir.AluOpType.mult)
                        # partition shift 0:64 -> 64:128 needs a DMA, not DVE
                        nc.sync.dma_start(aob[DH:P, qo:qo + qw],
                                          tmpb[:, qo:qo + qw])

        # ---- partial out-projection for one batch: [T, 1024] bf16 -> DRAM.
        def out_proj(b, use_scalar):
            for mt in range(math.ceil(MQ[b] / P)):
                pm = min(P, MQ[b] - mt * P)
                osb = sb.tile([P, D], BF16, tag="osb", bufs=3, name=f"osb{b}_{mt}")
                for nf in range(2):
                    ops = ps.tile([P, 512], F32, tag="ss", bufs=2,
                                  name=f"op{b}_{mt}_{nf}")
                    nc.tensor.matmul(ops[0:pm, :],
                                     aoT[b][:, mt * P: mt * P + pm],
                                     wout_sb[:, nf * 512:(nf + 1) * 512],
                                     start=True, stop=True, skip_group_check=True)
                    if nf == 1 and use_scalar:
                        # scalar engine is free of exp work by now
                        nc.scalar.activation(osb[0:pm, 512:D], ops[0:pm, :],
                                             mybir.ActivationFunctionType.Copy)
                    else:
                        nc.vector.tensor_copy(osb[0:pm, nf * 512:(nf + 1) * 512],
                                              ops[0:pm, :])
                dma(out_ext[b * T + mt * P: b * T + mt * P + pm, 0:512],
                    osb[0:pm, 0:512], seq=nc.sync)
                dma(out_ext[b * T + mt * P: b * T + mt * P + pm, 512:D],
                    osb[0:pm, 512:D], seq=nc.gpsimd)

        # ---- schedule (program order = scheduler priority)
        proj_kq(0)
        s_exp(0)       # streams on ACT while PE continues below
        proj_v(0)
        proj_kq(1)
        proj_v(1)
        s_exp(1)
        av_norm(0)     # runs under the exp(1) stream
        out_proj(0, use_scalar=False)   # fills the exp(1) tail
        av_norm(1)
        out_proj(1, use_scalar=True)

    nc.compile()
    return nc


_GRAPH_CACHE: dict = {}


def _get_graph(T: int, mq0: int, mq1: int):
    key = (T, mq0, mq1)
    if key not in _GRAPH_CACHE:
        _GRAPH_CACHE[key] = _build(T, mq0, mq1)
    return _GRAPH_CACHE[key]


def kernel(x, mask, W_qkv, W_out):
    x = np.asarray(x, dtype=np.float32)
    mask = np.asarray(mask, dtype=np.float32)
    W_qkv = np.asarray(W_qkv, dtype=np.float32)
    W_out = np.asarray(W_out, dtype=np.float32)
    b, n, d = x.shape
    assert (b, d) == (2, D) and W_qkv.shape == (D, 3 * D)

    idx = [np.nonzero(mask[i] > 0.5)[0] for i in range(b)]
    m = [len(ix) for ix in idx]
    nkt = max(1, math.ceil(max(m) / P))
    T = nkt * P
    mq = [min(T, max(16, math.ceil(mi / 16) * 16)) for mi in m]

    nc = _get_graph(T, mq[0], mq[1])

    bf16 = ml_dtypes.bfloat16
    xg = np.zeros((b, T, d), dtype=np.float32)
    for i in range(b):
        xg[i, :m[i]] = x[i][idx[i]]
    xt_all = np.ascontiguousarray(
        xg.reshape(b * T, d).transpose(1, 0)).astype(bf16)   # [D, 2T]
    npad = np.array([[T - m[0], T - m[1]]], dtype=np.float32)

    in_maps = []
    for c in range(N_CORES):
        cols = slice(c * P, (c + 1) * P)
        wqkv_c = np.ascontiguousarray(np.concatenate(
            [W_qkv[:, 0 * D:1 * D][:, cols],
             W_qkv[:, 1 * D:2 * D][:, cols],
             W_qkv[:, 2 * D:3 * D][:, cols]], axis=1)).astype(bf16)
        wout_c = np.ascontiguousarray(W_out[cols, :]).astype(bf16)
        in_maps.append({
            "xt": xt_all,
            "wqkv": wqkv_c,
            "wout": wout_c,
            "npad": npad,
        })

    res = run_bass_kernel_spmd(nc, in_maps, core_ids=list(range(N_CORES)))

    total = np.zeros((b * T, d), dtype=np.float32)
    for c in range(N_CORES):
        total += np.asarray(res.results[c]["out"], dtype=np.float32)

    out = np.zeros((b, n, d), dtype=np.float32)
    for i in range(b):
        out[i][idx[i]] = total[i * T: i * T + m[i]]
    return out
